# revision 76
# baseline (speedup 1.0000x reference)
"""Trainium2 Bass kernel for AttentionWithSpatial.

Computation (per batch b of 4, n=2048, dim=256, 4 heads x 64):
    qkv = x @ W_qkv ; split q,k,v; heads
    dots = (q @ k^T) * 64**-0.5 + spatial ;  masked (mask==0 -> -inf)
    attn = softmax(dots) ; out = (attn @ v) reshaped @ W_out + b_out

Sharding: 8 cores = 4 batches x 2 query-row halves (1024 rows each).

Host prep (elementwise only): ebT = exp(where(mask==0,-inf,spatial))^T in
fp16 (exp shift cancels in the final normalization), xT = x^T fp16, and
W_q pre-scaled by 64**-0.5.

On-core algorithm (transposed-score domain):
    dotsT[j,i] = k_h^T q_h matmul             PSUM f32 (q pre-scaled)
    ax = exp(dotsT - 8)                       ACT, fp16 (shift cancels)
    at = ax * ebT                             DVE, fp16
    [outT_h; sums_h] = [v_h | 1]^T @ at       PSUM f32 (ones row => sums)
    zps_h = outT_h^T @ W_out_h ; acc += zps_h / sums_h ; (+b_out) -> out
"""

import sys

if "/opt/trn_rl_repo" not in sys.path:
    sys.path.insert(0, "/opt/trn_rl_repo")

import numpy as np

B = 4
N = 2048
D = 256
H = 4
DH = 64
ROWS = N // 2          # query rows per core
NJT = N // 128         # 16 key tiles
SCALE = DH ** -0.5     # 0.125
CSHIFT = -8.0          # exp shift; cancels in normalization

_cache = {}


def _build_program():
    import concourse.bass as bass
    import concourse.mybir as mybir
    import concourse.tile as tile
    from concourse import bacc
    from concourse.masks import make_identity
    from contextlib import ExitStack

    f32 = mybir.dt.float32
    f16 = mybir.dt.float16
    AF = mybir.ActivationFunctionType
    OP = mybir.AluOpType

    nc = bacc.Bacc("TRN2", target_bir_lowering=False,
                   dynamic_dma_scratch_size=32768)

    xt = nc.dram_tensor("xt", [D, N], f16, kind="ExternalInput")
    xqt = nc.dram_tensor("xqt", [D, ROWS], f16, kind="ExternalInput")
    ebt = nc.dram_tensor("ebt", [N, ROWS], f16, kind="ExternalInput")
    wqkv = nc.dram_tensor("wqkv", [D, 3 * D], f16, kind="ExternalInput")
    wout = nc.dram_tensor("wout", [D, D], f16, kind="ExternalInput")
    bout = nc.dram_tensor("bout", [D], f32, kind="ExternalInput")
    out = nc.dram_tensor("out", [ROWS, D], f32, kind="ExternalOutput")

    xt_r = xt[:].rearrange("(kt p) j -> p kt j", p=128)
    xqt_r = xqt[:].rearrange("(kt p) i -> p kt i", p=128)
    ebt_r = ebt[:].rearrange("(g p) i -> p g i", p=128)
    wqkv_r = wqkv[:].rearrange("(kt p) f -> p kt f", p=128)
    wout_r = wout[:].rearrange("(a p) f -> p a f", p=64)

    with tile.TileContext(nc) as tc, ExitStack() as ctx:
        persist = ctx.enter_context(tc.tile_pool(name="persist", bufs=1))
        psD = ctx.enter_context(tc.tile_pool(name="psD", bufs=2, space="PSUM"))
        psA = ctx.enter_context(tc.tile_pool(name="psA", bufs=2, space="PSUM"))
        psZ = ctx.enter_context(tc.tile_pool(name="psZ", bufs=2, space="PSUM"))

        w_sb = persist.tile([128, 2, 3 * D], f16)
        wout_sb = persist.tile([64, H, D], f16)
        ident16 = persist.tile([128, 128], f16)
        badd = persist.tile([128, D], f32)
        cshift = persist.tile([128, 1], f32)
        # PE warmup: a back-to-back chain of junk transposes keeps PE
        # continuously busy from ~1us until the first real matmul (~4us),
        # so the pstate ramp (3us of continuous execution) is complete and
        # everything runs at full frequency. Outputs are never read.
        junk = persist.tile([128, 128], f16)
        nc.vector.memset(junk, 0.0)
        warm = psD.tile([128, 128], f16, tag="psd", name="warm")
        nc.tensor.transpose(warm[:, 0:128], junk, junk)
        nc.vector.memset(cshift, CSHIFT)
        xT_sb = persist.tile([128, 2, N], f16)
        xqT_sb = persist.tile([128, 2, ROWS], f16)
        qT_sb = persist.tile([128, 2, ROWS], f16)
        kT_sb = persist.tile([128, 2, N], f16)
        v_sb = persist.tile([128, NJT, H, DH + 1], f16)
        ebT_sb = persist.tile([128, NJT, ROWS], f16)
        make_identity(nc, ident16)

        # main-loop SBUF pools
        ax_pool = ctx.enter_context(tc.tile_pool(name="axp", bufs=10))
        at_pool = ctx.enter_context(tc.tile_pool(name="atp", bufs=8))
        o_pool = ctx.enter_context(tc.tile_pool(name="op", bufs=6))
        rs_pool = ctx.enter_context(tc.tile_pool(name="rsp", bufs=4))
        acc_pool = ctx.enter_context(tc.tile_pool(name="accp", bufs=8))
        os_pool = ctx.enter_context(tc.tile_pool(name="osp", bufs=6))

        # ------------- DMA issue order (priority = need time) -------------
        dma = nc.sync.dma_start
        dma(out=w_sb[:, :, 0:D + 128], in_=wqkv_r[:, :, 0:D + 128])  # q + k hp0
        dma(out=xqT_sb[:, :, 0:512], in_=xqt_r[:, :, 0:512])
        # xT j0:512 split in two: the first k matmul + kT copy (which gate
        # exp0 via the jt0 dots) start half a DMA earlier
        dma(out=xT_sb[:, :, 0:256], in_=xt_r[:, :, 0:256])
        dma(out=xT_sb[:, :, 256:512], in_=xt_r[:, :, 256:512])
        dma(out=ebT_sb[:, 0:2, :], in_=ebt_r[:, 0:2, :])
        dma(out=w_sb[:, :, 2 * D:3 * D], in_=wqkv_r[:, :, 2 * D:3 * D])  # v
        dma(out=xT_sb[:, :, 512:1024], in_=xt_r[:, :, 512:1024])
        dma(out=w_sb[:, :, D + 128:2 * D], in_=wqkv_r[:, :, D + 128:2 * D])
        dma(out=ebT_sb[:, 2:4, :], in_=ebt_r[:, 2:4, :])
        dma(out=xT_sb[:, :, 1024:1536], in_=xt_r[:, :, 1024:1536])
        dma(out=ebT_sb[:, 4:6, :], in_=ebt_r[:, 4:6, :])
        dma(out=xqT_sb[:, :, 512:1024], in_=xqt_r[:, :, 512:1024])
        dma(out=xT_sb[:, :, 1536:2048], in_=xt_r[:, :, 1536:2048])
        dma(out=ebT_sb[:, 6:8, :], in_=ebt_r[:, 6:8, :])
        dma(out=wout_sb, in_=wout_r)
        bout_ap = bout[:]
        dma(out=badd,
            in_=bass.AP(tensor=bout_ap.tensor, offset=bout_ap.offset,
                        ap=[[0, 128]] + list(bout_ap.ap)))
        for g in range(8, NJT, 4):
            dma(out=ebT_sb[:, g:g + 4, :], in_=ebt_r[:, g:g + 4, :])

        # ------------- prologue compute closures -------------
        def mm_k(hp, nch):
            def f():
                ps = psZ.tile([128, 512], f32, tag="zps", name=f"kps{hp}_{nch}")
                for kt in range(2):
                    nc.tensor.matmul(
                        ps, w_sb[:, kt, D + hp * 128:D + (hp + 1) * 128],
                        xT_sb[:, kt, nch * 512:(nch + 1) * 512],
                        start=(kt == 0), stop=(kt == 1))
                return ps
            return f

        def cp_k(hp, nch, ps, eng):
            eng.tensor_copy(kT_sb[:, hp, nch * 512:(nch + 1) * 512], ps)

        def mm_q(hp, c):
            ps = psZ.tile([128, 512], f32, tag="zps", name=f"qps{hp}_{c}")
            for kt in range(2):
                nc.tensor.matmul(
                    ps, w_sb[:, kt, hp * 128:(hp + 1) * 128],
                    xqT_sb[:, kt, c * 512:(c + 1) * 512],
                    start=(kt == 0), stop=(kt == 1))
            nc.vector.tensor_copy(qT_sb[:, hp, c * 512:(c + 1) * 512], ps)

        def mm_v(nt):
            def f():
                ps = psZ.tile([128, D], f32, tag="zps", name=f"vps{nt}")
                for kt in range(2):
                    nc.tensor.matmul(
                        ps, xT_sb[:, kt, nt * 128:(nt + 1) * 128],
                        w_sb[:, kt, 2 * D:3 * D],
                        start=(kt == 0), stop=(kt == 1))
                return ps
            return f

        def cp_v(nt, ps, eng):
            eng.tensor_copy(v_sb[:, nt, :, 0:DH],
                            ps.rearrange("p (h d) -> p h d", h=H))

        def emit_dots(c, hp, jt):
            # high priority: these feed the exp stream, which paces the
            # whole kernel -- they must win PE against AV/tail/prologue work
            psd = psD.tile([128, 1024], f32, tag="psd", name="psd")
            with tc.high_priority(offset=100000):
                for hh in range(2):
                    nc.tensor.matmul(
                        psd[:, hh * 512:(hh + 1) * 512],
                        kT_sb[hh * 64:(hh + 1) * 64, hp, jt * 128:(jt + 1) * 128],
                        qT_sb[hh * 64:(hh + 1) * 64, hp, c * 512:(c + 1) * 512],
                        start=True, stop=True)
            return psd

        # minimal serial prologue: enough for pass 0 jt0..3 to start.
        # q first (its DMA lands first); the k chain for j0:256 goes
        # through the idle ACT engine so it doesn't queue behind the qT
        # copy on DVE -- exp0 is gated by kT j0:128 + qT
        mm_q(0, 0)
        ka = psZ.tile([128, 256], f32, tag="zps", name="ka")
        for kt in range(2):
            nc.tensor.matmul(ka, w_sb[:, kt, D:D + 128],
                             xT_sb[:, kt, 0:256],
                             start=(kt == 0), stop=(kt == 1))
        nc.scalar.copy(kT_sb[:, 0, 0:256], ka)
        # dots for jt0/jt1 pre-emitted here: they only need kT j0:256 + qT,
        # and must sit before kb/v0 in the PE list so exp0 starts ASAP
        pre_dots = [emit_dots(0, 0, 0), emit_dots(0, 0, 1)]
        # kb/v0 use the avps slots (idle until pass 0's first AV): a psZ
        # ring wait here would let the scheduler order them after the
        # pre-dots' gates and head-of-line block the exp stream start
        kb = psA.tile([128, 256], f32, tag="avps", name="kb")
        for kt in range(2):
            nc.tensor.matmul(kb, w_sb[:, kt, D:D + 128],
                             xT_sb[:, kt, 256:512],
                             start=(kt == 0), stop=(kt == 1))
        nc.vector.tensor_copy(kT_sb[:, 0, 256:512], kb)
        v0 = psA.tile([128, D], f32, tag="avps", name="v0ps")
        for kt in range(2):
            nc.tensor.matmul(v0, xT_sb[:, kt, 0:128],
                             w_sb[:, kt, 2 * D:3 * D],
                             start=(kt == 0), stop=(kt == 1))
        cp_v(0, v0, nc.vector)
        nc.vector.memset(v_sb[:, :, :, DH:DH + 1], 1.0)

        # deferred prologue work, scheduled into pass-0/1 jt slots
        sched = {}

        def add(pi, jt, f):
            sched.setdefault((pi, jt), []).append(f)

        def kchain(hp, nch, eng):
            def f():
                ps = mm_k(hp, nch)()
                cp_k(hp, nch, ps, eng)
            return f

        def vchain(nt, eng):
            def f():
                ps = mm_v(nt)()
                cp_v(nt, ps, eng)
            return f

        add(0, 0, vchain(1, nc.vector))
        add(0, 0, kchain(0, 1, nc.vector))
        add(0, 0, vchain(2, nc.vector))
        add(0, 1, kchain(1, 0, nc.vector))
        add(0, 1, vchain(3, nc.vector))
        add(0, 2, lambda: mm_q(1, 0))
        add(0, 2, vchain(4, nc.vector))
        add(0, 3, vchain(5, nc.vector))
        add(0, 4, kchain(0, 2, nc.vector))
        add(0, 4, vchain(6, nc.vector))
        add(0, 5, vchain(7, nc.vector))
        add(0, 6, kchain(1, 1, nc.vector))
        add(0, 6, vchain(8, nc.vector))
        add(0, 7, vchain(9, nc.vector))
        add(0, 8, kchain(0, 3, nc.vector))
        add(0, 8, vchain(10, nc.vector))
        add(0, 9, vchain(11, nc.vector))
        add(0, 10, kchain(1, 2, nc.vector))
        add(0, 10, vchain(12, nc.vector))
        add(0, 11, vchain(13, nc.vector))
        add(0, 12, vchain(14, nc.vector))
        add(0, 13, kchain(1, 3, nc.vector))
        add(0, 13, vchain(15, nc.vector))
        add(1, 7, lambda: mm_q(0, 1))
        add(2, 7, lambda: mm_q(1, 1))

        # ------------- tail machinery -------------
        def tail_pieces(c, hp, o_pair, accs, drain=False):
            """Tail for pass (c, hp) as a list of closures; hp==1 finalizes.

            pss holds transposed row-sums: col 2*(itl*2+hh) of pss is the
            sums for (itl, hh); rs[:, itl*2+hh] is its reciprocal.
            """
            state = {}

            def head():
                # in the drain the avps (psA) slots are already free after
                # the o copies -- using one for pss keeps the psZ ring free
                # for the h3 zps tiles
                pss = (psA if drain else psZ).tile(
                    [128, 16], f16, tag="avps" if drain else "zps", name="pss")
                rs = rs_pool.tile([128, 8], f32, name="rs")
                for hh in range(2):
                    for itl in range(4):
                        k = itl * 2 + hh
                        nc.tensor.transpose(
                            pss[:, 2 * k:2 * k + 2],
                            o_pair[hh][DH:DH + 1, itl * 128:(itl + 1) * 128],
                            ident16[DH:DH + 1, DH:DH + 2])
                    if drain:
                        # per-hh reciprocal: hh0 STTs start before o1 lands
                        pv = pss.rearrange("p (k two) -> p k two", two=2)[:, :, 0]
                        pv = pv.rearrange("p (k h) -> p k h", h=2)[:, :, hh]
                        rv = rs.rearrange("p (k h) -> p k h", h=2)[:, :, hh]
                        with tc.high_priority(offset=50000):
                            nc.vector.reciprocal(rv, pv)
                if not drain:
                    nc.vector.reciprocal(
                        rs, pss.rearrange("p (k two) -> p k two", two=2)[:, :, 0])
                state["rs"] = rs

            def one_itl(itl):
                def f():
                    rs = state["rs"]
                    if hp == 0:
                        acc = acc_pool.tile([128, D], f16, name=f"acc{c}_{itl}",
                                            tag="acc")
                        accs[itl] = acc
                    elif drain or itl % 2 == 0:
                        state["osb2"] = os_pool.tile(
                            [128, 1 if drain else 2, D], f32, name="osb2")
                    acc = accs[itl]
                    for hh in range(2):
                        h = hp * 2 + hh
                        # during the drain the dots (psD) ring is free: put
                        # the h2 tiles there and the h3 tiles in psZ, so
                        # each ring's slot-reuse gate coincides with the itl
                        # chain's own acc dependency (no added serialization)
                        pool_, tag_ = (psD, "psd") if (drain and hh == 0) \
                            else (psZ, "zps")
                        zps = pool_.tile([128, D], f32, tag=tag_, name="zps")
                        nc.tensor.matmul(
                            zps, o_pair[hh][0:DH, itl * 128:(itl + 1) * 128],
                            wout_sb[:, h, :], start=True, stop=True)
                        eng = nc.vector
                        if h == 0:
                            eng.scalar_tensor_tensor(
                                out=acc, in0=zps,
                                scalar=rs[:, itl * 2 + hh:itl * 2 + hh + 1],
                                in1=badd, op0=OP.mult, op1=OP.add)
                        elif h == 3:
                            osb2 = state["osb2"]
                            eng.scalar_tensor_tensor(
                                out=osb2[:, 0 if drain else itl % 2, :],
                                in0=zps,
                                scalar=rs[:, itl * 2 + hh:itl * 2 + hh + 1],
                                in1=acc, op0=OP.mult, op1=OP.add)
                            if drain:
                                # unpaired: each itl's DMA fires the moment
                                # its own STT is done
                                lo = (c * 4 + itl) * 128
                                nc.sync.dma_start(out=out[lo:lo + 128, :],
                                                  in_=osb2[:, 0, :])
                            elif itl % 2 == 1:
                                # one DMA per itl pair: halves the HWDGE
                                # generation chain
                                lo = (c * 4 + itl - 1) * 128
                                dst = out[lo:lo + 256, :].rearrange(
                                    "(t p) d -> p t d", p=128)
                                nc.sync.dma_start(out=dst, in_=osb2)
                        else:
                            eng.scalar_tensor_tensor(
                                out=acc, in0=zps,
                                scalar=rs[:, itl * 2 + hh:itl * 2 + hh + 1],
                                in1=acc, op0=OP.mult, op1=OP.add)
                return f

            return [head] + [one_itl(i) for i in range(4)]

        # ------------- main passes -------------
        passes = [(0, 0), (0, 1), (1, 0), (1, 1)]
        accs_by_c = {0: [None] * 4, 1: [None] * 4}
        pending_tail = []

        for idx, (c, hp) in enumerate(passes):
            avps = [psA.tile([DH + 1, 512], f32, tag="avps", name=f"avps{hh}")
                    for hh in range(2)]
            ax2 = None
            for jt in range(NJT):
                psd = pre_dots.pop(0) if pre_dots else emit_dots(c, hp, jt)
                if idx == len(passes) - 1 and jt == NJT - 1:
                    # drain jt15: per-head exp/mult/AV so the final o copies
                    # start as early as possible; both exps emitted first so
                    # ACT runs them back-to-back
                    aths = []
                    for hh in range(2):
                        axh = ax2[:, 1024 + hh * 512:1024 + (hh + 1) * 512]
                        nc.scalar.activation(
                            axh, psd[:, hh * 512:(hh + 1) * 512], AF.Exp,
                            bias=cshift[:])
                        ath = at_pool.tile([128, 512], f16, name=f"ath{hh}")
                        aths.append((axh, ath))
                    for hh in range(2):
                        axh, ath = aths[hh]
                        nc.vector.tensor_mul(
                            ath, axh, ebT_sb[:, jt, c * 512:(c + 1) * 512])
                        nc.tensor.matmul(
                            avps[hh], v_sb[:, jt, hp * 2 + hh, :], ath,
                            start=False, stop=True, skip_group_check=True)
                    for f in sched.pop((idx, jt), []):
                        f()
                    continue
                if jt % 2 == 0:
                    ax2 = ax_pool.tile([128, 2048], f16, name="ax2")
                half = (jt % 2) * 1024
                nc.scalar.activation(ax2[:, half:half + 1024], psd, AF.Exp,
                                     bias=cshift[:])
                # the drain pass finishes its last two jts unpaired so the
                # final mult/AV/o chain is as short as possible
                single = (idx == len(passes) - 1 and jt >= NJT - 2)
                if jt % 2 == 1 or single:
                    if jt == NJT - 1 and idx + 1 < len(passes):
                        # fill the next pass's first dots tile while exp15
                        # still runs, so the exp stream crosses the pass
                        # boundary without a gap
                        nc2, nhp = passes[idx + 1]
                        pre_dots.append(emit_dots(nc2, nhp, 0))
                    jlo = jt if single else jt - 1
                    nsub = 1 if single else 2
                    at2 = at_pool.tile([128, 1024 * nsub], f16, name="at2")
                    eb = ebT_sb[:, jlo:jt + 1, c * 512:(c + 1) * 512]
                    ebap = list(eb.ap)
                    eb_rep = bass.AP(tensor=eb.tensor, offset=eb.offset,
                                     ap=[ebap[0], ebap[1], [0, 2], ebap[2]])
                    src = ax2[:, (jlo % 2) * 1024:(jt % 2) * 1024 + 1024]
                    nc.vector.tensor_mul(at2, src, eb_rep)
                    for hh in range(2):
                        for sub in range(nsub):
                            jj = jlo + sub
                            nc.tensor.matmul(
                                avps[hh], v_sb[:, jj, hp * 2 + hh, :],
                                at2[:, sub * 1024 + hh * 512:
                                    sub * 1024 + (hh + 1) * 512],
                                start=(jj == 0), stop=(jj == NJT - 1),
                                skip_group_check=True)
                for f in sched.pop((idx, jt), []):
                    f()
                if pending_tail and 2 <= jt <= 6:
                    pending_tail.pop(0)()
            o_pair = []
            for hh in range(2):
                o = o_pool.tile([DH + 1, 512], f16, name=f"o{hh}", tag="o")
                if idx == len(passes) - 1 and hh == 1:
                    # drain: o1 through the now-idle ACT engine, in
                    # parallel with o0 on DVE
                    nc.scalar.copy(o, avps[hh])
                else:
                    nc.vector.tensor_copy(o, avps[hh])
                o_pair.append(o)
            if idx == len(passes) - 1:
                for f in tail_pieces(c, hp, o_pair, accs_by_c[c], drain=True):
                    f()
            else:
                pending_tail.extend(
                    tail_pieces(c, hp, o_pair, accs_by_c[c]))
        for f in pending_tail:
            f()

    nc.compile()
    return nc


def _get_program():
    if "nc" not in _cache:
        _cache["nc"] = _build_program()
    return _cache["nc"]


def _make_in_maps(x, mask, spatial_weights, W_qkv, W_out, b_out):
    x = np.asarray(x, dtype=np.float32)
    mask = np.asarray(mask)
    spatial = np.asarray(spatial_weights, dtype=np.float32)
    wqkv16 = np.asarray(W_qkv).astype(np.float16)
    wqkv16[:, 0:D] *= np.float16(SCALE)
    wout16 = np.asarray(W_out).astype(np.float16)
    bo = np.ascontiguousarray(np.asarray(b_out, dtype=np.float32))
    in_maps = []
    for bi in range(B):
        eb = np.exp(spatial[bi]) * (mask[bi] != 0)      # [n_i, n_j] f32
        ebT = np.ascontiguousarray(eb.T.astype(np.float16))  # [j, i]
        xT = np.ascontiguousarray(x[bi].T.astype(np.float16))  # [d, n]
        for rh in range(2):
            rows = slice(rh * ROWS, (rh + 1) * ROWS)
            in_maps.append({
                "xt": xT,
                "xqt": np.ascontiguousarray(xT[:, rows]),
                "ebt": np.ascontiguousarray(ebT[:, rows]),
                "wqkv": wqkv16,
                "wout": wout16,
                "bout": bo,
            })
    return in_maps


def _run(in_maps, trace=False):
    from concourse.bass_utils import run_bass_kernel_spmd
    nc = _get_program()
    return run_bass_kernel_spmd(nc, in_maps, core_ids=list(range(8)), trace=trace)


def kernel(x, mask, spatial_weights, W_qkv, W_out, b_out):
    in_maps = _make_in_maps(x, mask, spatial_weights, W_qkv, W_out, b_out)
    res = _run(in_maps)
    full = np.empty((B, N, D), dtype=np.float32)
    for c in range(8):
        bi, rh = c // 2, c % 2
        full[bi, rh * ROWS:(rh + 1) * ROWS] = res.results[c]["out"]
    return full
